# revision 7
# baseline (speedup 1.0000x reference)
"""CosinePrediction edge-parallel kernel for 8 trn2 NeuronCores.

Strategy: shard the 1M edges across 8 cores (125k each). Per core, bucket
edges host-side by (src_quarter, dst_quarter) of the 100k-row tables —
16 buckets, capacity 8448, padded with index 0 — so row indices fit the
int16 index format of GPSIMD dma_gather. Each bucket does two dma_gather
ops (raw f32 rows of both tables, 4 SWDGE queues round-robin, double
buffered), then on-chip: s_uv = reduce(u*v), s_uu = reduce(u^2) (ACT
square + DVE reduce), s_vv likewise, and a tail computes
cos = s_uv * rsqrt(s_uu*s_vv). The host un-permutes bucket-sorted results.
"""
import sys
import os

sys.path.insert(0, "/opt/trn_rl_repo")

import numpy as np
from contextlib import ExitStack

N = 100_000          # rows per table
D = 64               # feature dim (256B rows)
E_TOTAL = 1_000_000
NCORES = 8
PER = E_TOTAL // NCORES     # 125000 edges per core
NQUART = 4
Q = N // NQUART             # 25000 rows per quarter (< int16 max)
NB = NQUART * NQUART        # 16 buckets per core
CAP = 8704                  # bucket capacity = 128 * 68 (div by 512)
BLK = CAP // 128            # 66 dst blocks
IDXC = CAP // 16            # 528 idx columns (wrapped in 16 partitions)
COLS = NB * BLK             # 1056 result columns

LAST_RESULT = None
_CACHED_NC = None


def _install_trace_shim():
    """Register the NTFF profile hook trn_boot couldn't (stub antenv), and
    neuter the S3 artifact upload. Only needed when BASS_TRACE=1."""
    try:
        import types
        if "antenv.axon_hooks" not in sys.modules:
            from trn_agent_boot.trn_boot import _ntff_profile_via_ctypes
            hook = _ntff_profile_via_ctypes("/opt/axon/libaxon_pjrt.so")
            mod = types.ModuleType("antenv.axon_hooks")
            mod.get_axon_ntff_profile_hook = lambda: hook
            mod.set_axon_ntff_profile_hook = lambda h: None
            sys.modules["antenv.axon_hooks"] = mod
            import antenv
            antenv.axon_hooks = mod
        import concourse.bass_utils as bu
        bu.upload_artifacts = lambda tmpdir: f"file://{tmpdir}"
    except Exception:
        pass


def build_nc():
    global _CACHED_NC
    if _CACHED_NC is not None:
        return _CACHED_NC
    import concourse.bass as bass
    import concourse.bacc as bacc
    import concourse.mybir as mybir
    from concourse.library_config import mlp

    f32 = mybir.dt.float32
    i16 = mybir.dt.int16

    nc = bacc.Bacc("TRN2", target_bir_lowering=False, debug=False,
                   num_swdge_queues=4, detect_race_conditions=False)
    hu = nc.dram_tensor("hu", [N, D], f32, kind="ExternalInput")
    hi = nc.dram_tensor("hi", [N, D], f32, kind="ExternalInput")
    idxu_d = nc.dram_tensor("idxu", [128, NB * IDXC], i16, kind="ExternalInput")
    idxv_d = nc.dram_tensor("idxv", [128, NB * IDXC], i16, kind="ExternalInput")
    out_d = nc.dram_tensor("out", [128, COLS], f32, kind="ExternalOutput")

    with ExitStack() as st:
        u = [st.enter_context(nc.sbuf_tensor(f"u{s}", [128, BLK, D], f32))
             for s in range(2)]
        v = [st.enter_context(nc.sbuf_tensor(f"v{s}", [128, BLK, D], f32))
             for s in range(2)]
        m = [st.enter_context(nc.sbuf_tensor(f"m{s}", [128, BLK, D], f32))
             for s in range(2)]
        w = [st.enter_context(nc.sbuf_tensor(f"w{s}", [128, BLK, D], f32))
             for s in range(2)]
        idxu = st.enter_context(nc.sbuf_tensor("idxu_sb", [128, NB * IDXC], i16))
        idxv = st.enter_context(nc.sbuf_tensor("idxv_sb", [128, NB * IDXC], i16))
        suv = st.enter_context(nc.sbuf_tensor("suv", [128, COLS], f32))
        suu = st.enter_context(nc.sbuf_tensor("suu", [128, COLS], f32))
        svv = st.enter_context(nc.sbuf_tensor("svv", [128, COLS], f32))

        S_idx = st.enter_context(nc.semaphore("S_idx"))
        S_q = [st.enter_context(nc.semaphore(f"S_q{q}")) for q in range(4)]
        S_suv = [st.enter_context(nc.semaphore(f"S_suv{s}")) for s in range(2)]
        S_usq = [st.enter_context(nc.semaphore(f"S_usq{s}")) for s in range(2)]
        S_vsq = [st.enter_context(nc.semaphore(f"S_vsq{s}")) for s in range(2)]
        S_mult = [st.enter_context(nc.semaphore(f"S_mult{s}")) for s in range(2)]
        S_suu = [st.enter_context(nc.semaphore(f"S_suu{s}")) for s in range(2)]
        S_t = st.enter_context(nc.semaphore("S_t"))
        S_sq = st.enter_context(nc.semaphore("S_sq"))
        S_res = st.enter_context(nc.semaphore("S_res"))
        S_out = st.enter_context(nc.semaphore("S_out"))

        block = st.enter_context(nc.Block())

        @block.sync
        def _(sync):
            sync.dma_start(idxu[:], idxu_d[:, :]).then_inc(S_idx, 16)
            sync.dma_start(idxv[:], idxv_d[:, :]).then_inc(S_idx, 16)
            sync.wait_ge(S_res, 1)
            sync.dma_start(out_d[:, :], suv[:]).then_inc(S_out, 16)
            sync.wait_ge(S_out, 16)

        @block.gpsimd
        def _(gpsimd):
            gpsimd.load_library(mlp)
            gpsimd.wait_ge(S_idx, 32)
            for b in range(NB):
                s, k = b % 2, b // 2
                qs, qd = b // NQUART, b % NQUART
                NH, H, HI = 4, CAP // 4, IDXC // 4
                BQ = BLK // 4
                if k >= 1:
                    gpsimd.wait_ge(S_mult[s], k)
                    gpsimd.wait_ge(S_usq[s], k)
                for h in range(NH):
                    gpsimd.dma_gather(
                        u[s][:, h * BQ:(h + 1) * BQ, :],
                        hu[qs * Q:(qs + 1) * Q, :],
                        idxu[:, b * IDXC + h * HI:b * IDXC + (h + 1) * HI],
                        H, H, D, single_packet=False, queue_num=h % 2,
                    ).then_inc(S_q[h % 2], 16)
                if k >= 1:
                    gpsimd.wait_ge(S_vsq[s], k)
                for h in range(NH):
                    gpsimd.dma_gather(
                        v[s][:, h * BQ:(h + 1) * BQ, :],
                        hi[qd * Q:(qd + 1) * Q, :],
                        idxv[:, b * IDXC + h * HI:b * IDXC + (h + 1) * HI],
                        H, H, D, single_packet=False, queue_num=2 + h % 2,
                    ).then_inc(S_q[2 + h % 2], 16)

        @block.vector
        def _(vector):
            for b in range(NB):
                s, k = b % 2, b // 2
                cols = slice(b * BLK, (b + 1) * BLK)
                for q in range(4):
                    vector.wait_ge(S_q[q], 32 * (b + 1))
                vector.tensor_tensor(out=m[s][:], in0=u[s][:], in1=v[s][:],
                                     op=mybir.AluOpType.mult
                                     ).then_inc(S_mult[s], 1)
                vector.tensor_reduce(out=suv[:, cols], in_=m[s][:],
                                     axis=mybir.AxisListType.X,
                                     op=mybir.AluOpType.add,
                                     ).then_inc(S_suv[s], 1)
                vector.wait_ge(S_usq[s], k + 1)
                vector.tensor_reduce(out=suu[:, cols], in_=w[s][:],
                                     axis=mybir.AxisListType.X,
                                     op=mybir.AluOpType.add,
                                     ).then_inc(S_suu[s], 1)
                vector.wait_ge(S_vsq[s], k + 1)
                vector.tensor_reduce(out=svv[:, cols], in_=m[s][:],
                                     axis=mybir.AxisListType.X,
                                     op=mybir.AluOpType.add)
            # tail: cos = s_uv * rsqrt(s_uu * s_vv)
            vector.tensor_tensor(out=suu[:], in0=suu[:], in1=svv[:],
                                 op=mybir.AluOpType.mult).then_inc(S_t, 1)
            vector.wait_ge(S_sq, 1)
            vector.reciprocal(out=suu[:], in_=svv[:])
            vector.tensor_tensor(out=suv[:], in0=suv[:], in1=suu[:],
                                 op=mybir.AluOpType.mult).then_inc(S_res, 1)

        @block.scalar
        def _(scalar):
            for b in range(NB):
                s, k = b % 2, b // 2
                scalar.wait_ge(S_q[0], 32 * (b + 1))
                scalar.wait_ge(S_q[1], 32 * (b + 1))
                if k >= 1:
                    scalar.wait_ge(S_suu[s], k)
                scalar.activation(out=w[s][:], in_=u[s][:],
                                  func=mybir.ActivationFunctionType.Square,
                                  ).then_inc(S_usq[s], 1)
                scalar.wait_ge(S_suv[s], k + 1)
                scalar.activation(out=m[s][:], in_=v[s][:],
                                  func=mybir.ActivationFunctionType.Square,
                                  ).then_inc(S_vsq[s], 1)
            scalar.wait_ge(S_t, 1)
            scalar.activation(out=svv[:], in_=suu[:],
                              func=mybir.ActivationFunctionType.Sqrt,
                              ).then_inc(S_sq, 1)

    nc.compile()
    _CACHED_NC = nc
    return nc


def _prep_core(src, dst):
    """Bucket one core's edges; returns (idxu, idxv, order, counts)."""
    qs = src // Q
    qd = dst // Q
    bucket = qs * NQUART + qd
    order = np.argsort(bucket, kind="stable")
    counts = np.bincount(bucket, minlength=NB)
    if counts.max() > CAP:
        raise RuntimeError(f"bucket overflow: {counts.max()} > {CAP}")
    su, du = src[order], dst[order]
    idxu = np.zeros((128, NB * IDXC), np.int16)
    idxv = np.zeros((128, NB * IDXC), np.int16)
    off = 0
    for b in range(NB):
        n = counts[b]
        lu = np.zeros(CAP, np.int64)
        lv = np.zeros(CAP, np.int64)
        lu[:n] = su[off:off + n] - (b // NQUART) * Q
        lv[:n] = du[off:off + n] - (b % NQUART) * Q
        off += n
        wu = lu.reshape(IDXC, 16).T.astype(np.int16)   # i -> (i%16, i//16)
        wv = lv.reshape(IDXC, 16).T.astype(np.int16)
        idxu[:, b * IDXC:(b + 1) * IDXC] = np.tile(wu, (8, 1))
        idxv[:, b * IDXC:(b + 1) * IDXC] = np.tile(wv, (8, 1))
    return idxu, idxv, order, counts


def kernel(h_user, h_item, src_idx, dst_idx):
    global LAST_RESULT
    from concourse.bass_utils import run_bass_kernel_spmd

    if os.environ.get("BASS_TRACE"):
        _install_trace_shim()

    hu = np.ascontiguousarray(np.asarray(h_user, dtype=np.float32))
    hi = np.ascontiguousarray(np.asarray(h_item, dtype=np.float32))
    src = np.asarray(src_idx).astype(np.int64)
    dst = np.asarray(dst_idx).astype(np.int64)
    idx_dtype = np.asarray(src_idx).dtype

    nc = build_nc()

    in_maps, metas = [], []
    for c in range(NCORES):
        s = src[c * PER:(c + 1) * PER]
        d = dst[c * PER:(c + 1) * PER]
        idxu, idxv, order, counts = _prep_core(s, d)
        in_maps.append({"hu": hu, "hi": hi, "idxu": idxu, "idxv": idxv})
        metas.append((order, counts))

    res = run_bass_kernel_spmd(nc, in_maps, core_ids=list(range(NCORES)))
    LAST_RESULT = res

    outs = []
    for c in range(NCORES):
        order, counts = metas[c]
        arr = res.results[c]["out"].reshape(128, NB, BLK)
        # slot i of bucket b lives at [i % 128, b, i // 128]
        arr2 = arr.transpose(1, 2, 0).reshape(NB, CAP)
        cos_sorted = np.concatenate(
            [arr2[b, :counts[b]] for b in range(NB)])
        res_core = np.empty(PER, np.float32)
        res_core[order] = cos_sorted
        outs.append(res_core)
    out = np.concatenate(outs).reshape(E_TOTAL, 1).astype(np.float32)
    # keep index inputs' dtype untouched; output is f32 like the reference
    del idx_dtype
    return out
